# revision 8
# baseline (speedup 1.0000x reference)
"""Trainium2 Bass kernel: batched 4-point DLT homography (closed-form solve).

Contract: kernel(pts_1_tile, pred_h4p_tile) -> [B, 3, 3] float32, with
B = 524288 split across 8 NeuronCores (batch-parallel, no communication).

Math (per batch element, points p=0..3 with src (x_p,y_p), dst (X_p,Y_p)):
the DLT system rows are
    x h0 + y h1 + h2 = X (1 + x h6 + y h7)
    x h3 + y h4 + h5 = Y (1 + x h6 + y h7)
Eliminating (h0,h1,h2) from the four X-equations via the left null vector n
of M = [(x_p, y_p, 1)] gives one linear equation in (h6,h7); same for the
Y-equations. Solve the 2x2, back out the rest in closed form.

Layout: per-core 65536 elements as [128 partitions, 512 free], processed in
two asymmetric chunks (128 + 384 columns) so chunk-0 compute starts as soon
as a quarter of the input has landed.  I/O is fp16 (host casts): halves DMA
time both ways.  Every per-element scalar is a [128, fc] "plane"; planes
live at fixed offsets in slabs so related planes are contiguous and steps
fuse into multi-plane single instructions.  ScalarE does interleave<->planar
shuffles; VectorE and GPSIMD split the elementwise math via a greedy
balancer calibrated from a HW trace (DVE fp16 0.52 ns/elem, Pool ~1.98);
the serial spine (diffs -> n -> 2x2 -> recip -> h67) is pinned to VectorE.
Reciprocals of n3 and det are fused into one two-plane fp32 op.
"""
import sys

for _p in ("/opt/trn_rl_repo", "/root/.axon_site/_ro/trn_rl_repo"):
    if _p not in sys.path:
        sys.path.append(_p)

import numpy as np

import concourse.bass as bass
import concourse.mybir as mybir
from concourse import bacc
from concourse.tile import TileContext
from concourse.bass_utils import run_bass_kernel_spmd

N_CORES = 8
B_TOTAL = 524288
PER_CORE = B_TOTAL // N_CORES  # 65536
PARTS = 128
F = PER_CORE // PARTS  # 512
CHUNKS = [128, 384]
FP32 = mybir.dt.float32
FP16 = mybir.dt.float16

ADD = mybir.AluOpType.add
SUB = mybir.AluOpType.subtract
MUL = mybir.AluOpType.mult


class _Slab:
    """Bump allocator with explicit free, in F-plane units, first-fit."""

    def __init__(self, nplanes):
        self.free = [(0, nplanes)]
        self.nplanes = nplanes

    def alloc(self, n):
        for idx, (off, ln) in enumerate(self.free):
            if ln >= n:
                if ln == n:
                    self.free.pop(idx)
                else:
                    self.free[idx] = (off + n, ln - n)
                return off
        raise RuntimeError(f"slab OOM: need {n}, free={self.free}")

    def release(self, off, n):
        self.free.append((off, n))
        self.free.sort()
        merged = []
        for o, ln in self.free:
            if merged and merged[-1][0] + merged[-1][1] == o:
                merged[-1] = (merged[-1][0], merged[-1][1] + ln)
            else:
                merged.append([o, ln])
        self.free = [tuple(m) if isinstance(m, list) else m for m in merged]


class _Bal:
    """Greedy VectorE/GPSIMD balance by measured op time (ns).

    DVE: 0.52 ns/elem fp16-2x (contiguous), ~105 ns fixed.
    Pool: 1.98 ns/elem (0.42 impl efficiency), ~240 ns fixed.
    """

    def __init__(self, nc):
        self.nc = nc
        self.t_v = 0.0
        self.t_g = 0.0

    def cv(self, fd, onex=False):
        rate = 1.04 if onex else 0.52
        return fd * rate + 105.0

    def cg(self, fd):
        return fd * 1.98 + 240.0

    def pick(self, fd, pin, onex=False):
        cv, cg = self.cv(fd, onex), self.cg(fd)
        if pin is None:
            eng = "v" if self.t_v + cv <= self.t_g + cg else "g"
        else:
            eng = pin
        if eng == "v":
            self.t_v += cv
            return self.nc.vector
        self.t_g += cg
        return self.nc.gpsimd


def _fd(ap):
    n = 1
    for d in ap.shape[1:]:
        n *= d
    return n


OPLOG = {}


def _build():
    OPLOG.clear()
    nc = bacc.Bacc(None, target_bir_lowering=False, debug=True)
    pts = nc.dram_tensor("pts", [PER_CORE, 8], FP16, kind="ExternalInput")
    prd = nc.dram_tensor("prd", [PER_CORE, 8], FP16, kind="ExternalInput")
    out = nc.dram_tensor("out", [PER_CORE, 9], FP16, kind="ExternalOutput")

    N32 = 6   # fp32 slab: [n3_32, det_32, rD_32, rdet_32, scratch x2]
    NP = 60   # fp16 compute-plane slab

    with TileContext(nc) as tc:
        with tc.tile_pool(name="st", bufs=1) as pool:
            tiles = {}
            for c, fc in enumerate(CHUNKS):
                tiles[c] = {
                    "vt": pool.tile([PARTS, 8 * fc], FP16, tag=f"vt{c}", name=f"vt{c}"),
                    "pt": pool.tile([PARTS, 8 * fc], FP16, tag=f"pt{c}", name=f"pt{c}"),
                    "ut": pool.tile([PARTS, 8 * fc], FP16, tag=f"ut{c}", name=f"ut{c}"),
                    "ot": pool.tile([PARTS, 9 * fc], FP16, tag=f"ot{c}", name=f"ot{c}"),
                    "s32": pool.tile([PARTS, N32 * fc], FP32, tag=f"s32_{c}", name=f"s32_{c}"),
                    "sp": pool.tile([PARTS, NP * fc], FP16, tag=f"sp{c}", name=f"sp{c}"),
                }

            # All input DMAs up front, chunk 0 first (smallest => earliest start)
            lo = 0
            for c, fc in enumerate(CHUNKS):
                hi = lo + PARTS * fc
                nc.sync.dma_start(
                    out=tiles[c]["vt"][:, :],
                    in_=pts[lo:hi, :].rearrange("(p f) c -> p (f c)", p=PARTS),
                )
                nc.sync.dma_start(
                    out=tiles[c]["pt"][:, :],
                    in_=prd[lo:hi, :].rearrange("(p f) c -> p (f c)", p=PARTS),
                )
                lo = hi

            lo = 0
            for c, fc in enumerate(CHUNKS):
                hi = lo + PARTS * fc
                vt, pt, ut, ot = (tiles[c][k] for k in ("vt", "pt", "ut", "ot"))
                slab32, slabp = tiles[c]["s32"], tiles[c]["sp"]
                sa = _Slab(NP)
                bal = _Bal(nc)
                last = c == len(CHUNKS) - 1

                def R32(off, n):
                    return slab32[:, off * fc : (off + n) * fc]

                def R(off, n):
                    return slabp[:, off * fc : (off + n) * fc]

                def V(off, n):
                    return R(off, n).rearrange("p (c f) -> p c f", f=fc)

                def PL(off):
                    return R(off, 1)

                def BC(off, k):
                    return PL(off).unsqueeze(1).broadcast_to((PARTS, k, fc))

                def tt(o, a, b, op, pin=None, onex=False, desc=""):
                    eng = bal.pick(_fd(o), pin, onex)
                    ins = eng.tensor_tensor(out=o, in0=a, in1=b, op=op)
                    OPLOG[ins.ins.name] = desc or "tt"

                def stt(o, in0, scalar, in1, op0, op1, desc="stt"):
                    bal.t_v += bal.cv(_fd(o))
                    ins = nc.vector.scalar_tensor_tensor(
                        out=o, in0=in0, scalar=scalar, in1=in1, op0=op0, op1=op1
                    )
                    OPLOG[ins.ins.name] = desc

                def scp(o, i, desc="scp"):
                    ins = nc.scalar.copy(out=o, in_=i)
                    OPLOG[ins.ins.name] = desc

                # u = v + pred (interleaved fp16, 2 element-halves so the
                # u-deint pieces can start early)
                half = 4 * fc
                tt(ut[:, :half], vt[:, :half], pt[:, :half], ADD, pin="v",
                   desc="uaddV")
                tt(ut[:, half:], vt[:, half:], pt[:, half:], ADD, pin="v",
                   desc="uaddV2")

                # deinterleave: comp (0,2,4,6 / 1,3,5,7) -> planar
                xv = sa.alloc(8)  # [x0,x1,x2,x3,y0,y1,y2,y3]
                uu = sa.alloc(8)  # [X0,X1,X2,X3,Y0,Y1,Y2,Y3]
                iv = vt[:, :].rearrange("p (f c g) -> p g c f", c=4, g=2)
                ov_ = R(xv, 8).rearrange("p (g c f) -> p g c f", c=4, g=2)
                scp(ov_[:, 0, :, :], iv[:, 0, :, :], desc="deint_vx")
                scp(ov_[:, 1, :, :], iv[:, 1, :, :], desc="deint_vy")
                iu = ut[:, :].rearrange("p (f c g) -> p g c f", c=4, g=2)
                ou_ = R(uu, 8).rearrange("p (g c f) -> p g c f", c=4, g=2)
                hf = fc // 2
                scp(ou_[:, :, :, :hf], iu[:, :, :, :hf], desc="deint_u1")
                scp(ou_[:, :, :, hf:], iu[:, :, :, hf:], desc="deint_u2")

                # OT is element-interleaved (f*9 + c): out-DMA is contiguous
                ov = ot[:, :].rearrange("p (f c) -> p c f", c=9)
                ins = nc.gpsimd.memset(ov[:, 8, :], 1.0)
                OPLOG[ins.ins.name] = "ones_g"

                # diffs: D = [dx1,dx2,dx3,dy1,dy2,dy3]
                dd = sa.alloc(6)
                xv3 = V(xv, 8)
                tt(V(dd, 6)[:, 0:3, :], xv3[:, 1:4, :], BC(xv, 3), SUB,
                   pin="v", desc="diffx")
                tt(V(dd, 6)[:, 3:6, :], xv3[:, 5:8, :], BC(xv + 4, 3), SUB,
                   pin="v", desc="diffy")
                DX1, DX2, DX3, DY1, DY2, DY3 = range(dd, dd + 6)

                # n: n1=dx2dy3-dx3dy2, n2=dx3dy1-dx1dy3, n3=dx1dy2-dx2dy1
                pa = sa.alloc(3)
                pb = sa.alloc(3)
                for k, (a, b) in enumerate(((DX2, DY3), (DX3, DY1), (DX1, DY2))):
                    tt(PL(pa + k), PL(a), PL(b), MUL, pin="v", desc=f"pa{k}")
                for k, (a, b) in enumerate(((DX3, DY2), (DX1, DY3), (DX2, DY1))):
                    tt(PL(pb + k), PL(a), PL(b), MUL, pin="v", desc=f"pb{k}")
                ns = sa.alloc(4)  # fp16 [n0,n1,n2,n3]
                tt(R(ns + 1, 3), R(pa, 3), R(pb, 3), SUB, pin="v", desc="nsub")
                # fp32 n3 for the reciprocal (sub done at fp32 from fp16 in)
                tt(R32(0, 1), PL(pa + 2), PL(pb + 2), SUB, pin="v", onex=True,
                   desc="n3_32")
                t0 = sa.alloc(1)
                tt(PL(t0), PL(ns + 1), PL(ns + 2), ADD, pin="v", desc="t0")
                stt(PL(ns), PL(t0), -1.0, PL(ns + 3), MUL, SUB)  # n0=-(n1+n2)-n3
                sa.release(pa, 3)
                sa.release(pb, 3)
                sa.release(t0, 1)

                # dots, grouped by point p: ZW[3p..] = (z_p, z_p x_p, z_p y_p)
                zx = sa.alloc(12)
                zy = sa.alloc(12)
                for zz, w in ((zx, 0), (zy, 4)):
                    # r goes to GPSIMD when its consumer (TX/TY) is far
                    # enough down the in-order V queue to hide Pool latency
                    rpin = "g" if (not last or w == 0) else "v"
                    tt(V(zz, 12)[:, 0:12:3, :], V(ns, 4), V(uu, 8)[:, w : w + 4, :],
                       MUL, pin="v", desc=f"z{w}")
                    tt(V(zz, 12)[:, 1:12:3, :], V(zz, 12)[:, 0:12:3, :],
                       V(xv, 8)[:, 0:4, :], MUL, pin="v", desc=f"q{w}")
                    tt(V(zz, 12)[:, 2:12:3, :], V(zz, 12)[:, 0:12:3, :],
                       V(xv, 8)[:, 4:8, :], MUL, pin=rpin, desc=f"r{w}")
                tx = sa.alloc(6)
                tt(R(tx, 6), R(zx, 6), R(zx + 6, 6), ADD, pin="v", desc="TX")
                sa.release(zx, 12)
                ty = sa.alloc(6)
                tt(R(ty, 6), R(zy, 6), R(zy + 6, 6), ADD, pin="v", desc="TY")
                sa.release(zy, 12)
                ss = sa.alloc(6)  # [aX,bX,cX,aY,bY,cY]
                tt(R(ss, 3), R(tx, 3), R(tx + 3, 3), ADD, pin="v", desc="ssX")
                tt(R(ss + 3, 3), R(ty, 3), R(ty + 3, 3), ADD, pin="v", desc="ssY")
                sa.release(tx, 6)
                sa.release(ty, 6)

                # 2x2: det = bXcY-bYcX, h6n = cXaY-cYaX, h7n = bYaX-bXaY
                AX, BX, CX, AY, BY, CY = range(ss, ss + 6)
                pc = sa.alloc(3)
                pd = sa.alloc(3)
                for k, (a, b) in enumerate(((BX, CY), (CX, AY), (BY, AX))):
                    tt(PL(pc + k), PL(a), PL(b), MUL, pin="v", desc=f"pc{k}")
                for k, (a, b) in enumerate(((BY, CX), (CY, AX), (BX, AY))):
                    tt(PL(pd + k), PL(a), PL(b), MUL, pin="v", desc=f"pd{k}")
                # det at fp32 (adjacent to n3_32), h6n/h7n at fp16
                tt(R32(1, 1), PL(pc), PL(pd), SUB, pin="v", onex=True,
                   desc="det32")
                dt67 = sa.alloc(2)
                tt(V(dt67, 2), V(pc, 3)[:, 1:3, :], V(pd, 3)[:, 1:3, :], SUB,
                   pin="v", desc="dt67")
                sa.release(pc, 3)
                sa.release(pd, 3)
                sa.release(ss, 6)

                # fused reciprocal over [n3_32, det_32] -> [rD_32, rdet_32]
                nc.vector.reciprocal_approx_accurate(
                    out=R32(2, 2), in_=R32(0, 2), scratch=R32(4, 2)
                )
                bal.t_v += 2 * (2 * fc * 1.04 + 151.0)
                h67 = sa.alloc(2)
                rdetb = R32(3, 1).unsqueeze(1).broadcast_to((PARTS, 2, fc))
                tt(V(h67, 2), V(dt67, 2), rdetb, MUL, pin="v", onex=True,
                   desc="h67")
                scp(ov[:, 6:8, :], V(h67, 2), desc="h67cp")
                sa.release(dt67, 2)

                # rD -> fp16 for the hg multiplies
                rd = sa.alloc(1)
                scp(PL(rd), R32(2, 1), desc="rdcast")

                # XW_p = X_p (1 + x_p h6 + y_p h7), p=0..2; same for YW
                m1 = sa.alloc(3)
                m2 = sa.alloc(3)
                sp = sa.alloc(3)
                xw = sa.alloc(6)  # [XW0,XW1,XW2,YW0,YW1,YW2]
                tt(V(m1, 3), V(xv, 8)[:, 0:3, :], BC(h67, 3), MUL, pin="v",
                   desc="m1")
                tt(V(m2, 3), V(xv, 8)[:, 4:7, :], BC(h67 + 1, 3), MUL, pin="v",
                   desc="m2")
                tt(R(sp, 3), R(m1, 3), R(m2, 3), ADD, pin="v", desc="sp")
                # w = sp + 1 on ScalarE (frees DVE), reuse m1 as w
                ins = nc.scalar.add(out=R(m1, 3), in_=R(sp, 3), add=1.0)
                OPLOG[ins.ins.name] = "wp_s"
                tt(V(xw, 6)[:, 0:3, :], V(m1, 3), V(uu, 8)[:, 0:3, :], MUL,
                   pin="v", desc="XW")
                tt(V(xw, 6)[:, 3:6, :], V(m1, 3), V(uu, 8)[:, 4:7, :], MUL,
                   pin="v", desc="YW")
                sa.release(m1, 3)
                sa.release(m2, 3)
                sa.release(sp, 3)
                sa.release(h67, 2)
                sa.release(uu, 8)

                # PQ = (XW1-XW0, XW2-XW0, YW1-YW0, YW2-YW0)
                pq = sa.alloc(4)
                xwv = R(xw, 6).rearrange("p (a b f) -> p a b f", a=2, b=3)
                tt(
                    R(pq, 4).rearrange("p (a b f) -> p a b f", a=2, b=2),
                    xwv[:, :, 1:3, :],
                    xwv[:, :, 0, :].unsqueeze(2).broadcast_to((PARTS, 2, 2, fc)),
                    SUB,
                    pin="v",
                    desc="PQ",
                )

                # pE = (P1 dy2, Q1 dy2, dx1 P2, dx1 Q2)
                # pF = (P2 dy1, Q2 dy1, dx2 P1, dx2 Q1)
                pe = sa.alloc(4)
                pf = sa.alloc(4)
                pqv = V(pq, 4)
                tt(V(pe, 4)[:, 0:2, :], pqv[:, 0:3:2, :], BC(DY2, 2), MUL,
                   pin="v", desc="pe01")
                tt(V(pe, 4)[:, 2:4, :], pqv[:, 1:4:2, :], BC(DX1, 2), MUL,
                   pin="v", desc="pe23")
                tt(V(pf, 4)[:, 0:2, :], pqv[:, 1:4:2, :], BC(DY1, 2), MUL,
                   pin="v", desc="pf01")
                tt(V(pf, 4)[:, 2:4, :], pqv[:, 0:3:2, :], BC(DX2, 2), MUL,
                   pin="v", desc="pf23")
                hn = sa.alloc(4)  # [h0n, h3n, h1n, h4n]
                tt(R(hn, 4), R(pe, 4), R(pf, 4), SUB, pin="v", desc="hn")
                hg = sa.alloc(4)  # [h0, h3, h1, h4]
                tt(V(hg, 4), V(hn, 4), BC(rd, 4), MUL, pin="v", desc="hg")
                sa.release(pe, 4)
                sa.release(pf, 4)
                sa.release(hn, 4)
                sa.release(pq, 4)
                sa.release(rd, 1)

                scp(ov[:, 0:4:3, :], V(hg, 2), desc="hcopy")
                scp(ov[:, 1:5:3, :], V(hg + 2, 2), desc="hcopy2")

                # h2 = XW0 - x0 h0 - y0 h1 ; h5 = YW0 - x0 h3 - y0 h4
                ee = sa.alloc(4)  # (x0 h0, y0 h1, x0 h3, y0 h4)
                xy0 = V(xv, 8)[:, 0:5:4, :]  # (x0, y0)
                hgv = V(hg, 4)
                tt(V(ee, 4)[:, 0:2, :], xy0, hgv[:, 0:3:2, :], MUL, pin="v",
                   desc="ee1")
                tt(V(ee, 4)[:, 2:4, :], xy0, hgv[:, 1:4:2, :], MUL, pin="v",
                   desc="ee2")
                # On chunk 0 the s1->h25 tail feeds only ScalarE + out-DMA
                # (not the V queue), so Pool latency there is free.
                tailpin = "v" if last else "g"
                s1 = sa.alloc(2)
                eev = V(ee, 4)
                tt(V(s1, 2), V(xw, 6)[:, 0:4:3, :], eev[:, 0:3:2, :], SUB,
                   pin=tailpin, desc="s1")
                h25 = sa.alloc(2)
                tt(V(h25, 2), V(s1, 2), eev[:, 1:4:2, :], SUB, pin=tailpin,
                   desc="h25")
                scp(ov[:, 2:6:3, :], V(h25, 2), desc="h25cp")
                sa.release(ee, 4)
                sa.release(s1, 2)
                sa.release(hg, 4)
                sa.release(xw, 6)
                sa.release(dd, 6)
                sa.release(xv, 8)
                sa.release(h25, 2)
                sa.release(ns, 4)

                nc.sync.dma_start(
                    out=out[lo:hi, :].rearrange("(p f) c -> p (f c)", p=PARTS),
                    in_=ot[:, :],
                )
                lo = hi
    nc.finalize()
    return nc


_NC_CACHE = {}


def _get_nc():
    if "nc" not in _NC_CACHE:
        _NC_CACHE["nc"] = _build()
    return _NC_CACHE["nc"]


def kernel(pts_1_tile, pred_h4p_tile, _trace=False):
    pts = np.ascontiguousarray(
        np.asarray(pts_1_tile).reshape(B_TOTAL, 8).astype(np.float16)
    )
    prd = np.ascontiguousarray(
        np.asarray(pred_h4p_tile).reshape(B_TOTAL, 8).astype(np.float16)
    )
    nc = _get_nc()
    in_maps = [
        {
            "pts": pts[i * PER_CORE : (i + 1) * PER_CORE],
            "prd": prd[i * PER_CORE : (i + 1) * PER_CORE],
        }
        for i in range(N_CORES)
    ]
    res = run_bass_kernel_spmd(nc, in_maps, list(range(N_CORES)), trace=_trace)
    outs = np.concatenate([res.results[i]["out"] for i in range(N_CORES)], axis=0)
    H = outs.astype(np.float32).reshape(B_TOTAL, 3, 3)
    if _trace:
        return H, res
    return H
